# revision 7
# baseline (speedup 1.0000x reference)
"""FDS smooth kernel for Trainium2 (8 NeuronCores, data-parallel).

Math: out[i,:] = features[i,:] * S[b_i,:] + B[b_i,:]
  S = sqrt(clip(v2/v1, 0.1, 10))  (1.0 where v1 <= 0)
  B = m2 - m1*S                   (0.0 where v1 <= 0)
S/B are tiny [100,128] per-bucket tables precomputed on host (f16) and
replicated to every core.  Per 512-sample compute group on device:
  PE:   diff2[k,i] = (b_i - k)^2 via K=128 f16 matmul (exact hi/lo b^2)
  ACT:  oh = Relu(1 - diff2)  -> exact one-hot, f16
  PE:   per 128-sample tile: psum = oh_tile.T @ [S||B]  (f16 matmul)
  DVE:  out = f * Sg + Bg   (two tensor_tensor ops on strided PSUM views)
  DMA:  loads on sync (SP HWDGE ring), stores on scalar (ACT ring)

HBM layout per core (62500 samples, zero padding traffic):
  30 supers x 2048 samples, stripe 16/partition -> 8KB descriptors
  1 mini of 1024, stripe 8 -> 4KB descriptors
  1 tail of 36, stripe 4 on partitions 0..8
Stores are issued per 1024-sample pair (4KB descriptors).
"""

import os
import sys
import types

import bass_rust
import ml_dtypes
import numpy as np

import concourse.bass as bass
import concourse.mybir as mybir
from concourse.bass_types import AP
from concourse.bass_utils import run_bass_kernel_spmd
from concourse.tile import TileContext

# This walrus build accepts at most one semaphore wait per instruction.
WAIT_LIMIT = 1


def split_waits(nc, maxw=WAIT_LIMIT):
    """Move excess sem waits onto standalone same-engine carriers
    inserted immediately before the over-limit instruction."""
    n = 0
    for fn in nc.m.functions:
        for blk in fn.blocks:
            insts = blk.instructions
            if not any(
                i.sync_info is not None and len(i.sync_info.on_wait) > maxw
                for i in insts
            ):
                continue
            newl = []
            for ins in insts:
                si = ins.sync_info
                if si is not None and len(si.on_wait) > maxw:
                    waits = list(si.on_wait)
                    extra, keep = waits[:-maxw], waits[-maxw:]
                    while extra:
                        chunk, extra = extra[:maxw], extra[maxw:]
                        # EventSemaphore = sequencer-level wait carrier that
                        # does NOT flush the engine pipeline (a Drain would).
                        d = bass_rust.InstEventSemaphore(
                            name=f"WSPL-{nc.next_id()}", ins=[], outs=[]
                        )
                        d.engine = ins.engine
                        d.sync_info = mybir.SyncInfo(on_wait=chunk, on_update=[])
                        newl.append(d)
                        n += 1
                    ins.sync_info = mybir.SyncInfo(
                        on_wait=keep, on_update=list(si.on_update)
                    )
                newl.append(ins)
            blk.instructions = newl
    return n


N = 500_000
D = 128
NB = 100          # buckets
NCORES = 8
CLIP_MIN = 0.1
CLIP_MAX = 10.0

PER = N // NCORES             # 62500 samples per core
GROUP = 512                   # samples per compute group
SUPER = 2048                  # samples per feature-load DMA (1MB f32)
NSUPER = PER // SUPER         # 30 full supers
MINI = 1024                   # one mid chunk
TAILN = PER - NSUPER * SUPER - MINI   # 36
TAILP = TAILN // 4            # 9 partitions used by the tail group
BROWS = 3                     # b, hi(b^2), lo(b^2)
BCHUNK = 8192                 # one-hot slots per bucket-row DMA chunk

F32 = mybir.dt.float32
F16 = mybir.dt.float16

LAST_RESULTS = None           # test harness reads exec_time_ns off this


def _ensure_ntff_shim():
    """If BASS_TRACE is set but the image's antenv lacks axon_hooks,
    run_bass_kernel_spmd(trace=True) would die on import.  Provide the
    hook (via trn_agent_boot's ctypes path) or a None stub."""
    try:
        import antenv.axon_hooks  # noqa: F401
        return
    except ImportError:
        pass
    hook = None
    try:
        from trn_agent_boot.trn_boot import _ntff_profile_via_ctypes

        hook = _ntff_profile_via_ctypes("/opt/axon/libaxon_pjrt.so")
    except Exception:
        hook = None
    mod = types.ModuleType("antenv.axon_hooks")
    mod.get_axon_ntff_profile_hook = lambda: hook
    mod.set_axon_ntff_profile_hook = lambda h: None
    sys.modules["antenv.axon_hooks"] = mod
    try:
        import concourse.bass_utils as _bu

        _bu.upload_artifacts = lambda tmpdir: f"local://{tmpdir}"
    except Exception:
        pass


_ensure_ntff_shim()


def _regions():
    """Per-core sample regions: (base, nsamples, stripe, ngroups, parts)."""
    regs = []
    for s in range(NSUPER):
        regs.append((s * SUPER, SUPER, 16, SUPER // GROUP, 128))
    regs.append((NSUPER * SUPER, MINI, 8, MINI // GROUP, 128))
    regs.append((NSUPER * SUPER + MINI, TAILN, 4, 1, TAILP))
    return regs


REGIONS = _regions()
NGROUPS = sum(r[3] for r in REGIONS)          # 123
NSLOT = NGROUPS * GROUP                       # 62976 one-hot slots


def build_program():
    nc = bass.Bass("TRN2", debug=False)

    feat = nc.dram_tensor("feat", [PER, D], F32, kind="ExternalInput")
    # rows: b, hi(b^2), lo(b^2)  (f16), in permuted slot order
    brow = nc.dram_tensor("brow", [BROWS, NSLOT], F16, kind="ExternalInput")
    # cols k: psd[k,i] = -2k*b + b2hi + b2lo ; the k^2 term rides in the
    # per-partition activation bias (1 - k^2), keeping bt rows 3-127 zero.
    dif_w = nc.dram_tensor("dif_w", [128, 128], F16, kind="ExternalInput")
    kbias = nc.dram_tensor("kbias", [128, 1], F32, kind="ExternalInput")
    # [S||B] f16; rows 100-127 zero.
    sbt = nc.dram_tensor("sbt", [128, 2 * D], F16, kind="ExternalInput")
    outp = nc.dram_tensor("outp", [PER, D], F32, kind="ExternalOutput")

    # flat group schedule: (region_idx, local_group)
    sched = []
    for ri, (base, n, stripe, ngr, parts) in enumerate(REGIONS):
        for gl in range(ngr):
            sched.append((ri, gl))

    with TileContext(nc) as tc:
        with (
            tc.tile_pool(name="const", bufs=1) as cpool,
            tc.tile_pool(name="fin", bufs=3) as fpool,
            tc.tile_pool(name="bin", bufs=2) as bpool,
            tc.tile_pool(name="onehot", bufs=3) as opool,
            tc.tile_pool(name="mid", bufs=2) as mpool,
            tc.tile_pool(name="res", bufs=4) as rpool,
            tc.tile_pool(name="psd", bufs=3, space="PSUM") as psdpool,
            tc.tile_pool(name="psg", bufs=2, space="PSUM") as psgpool,
        ):
            dw_t = cpool.tile([128, 128], F16)
            nc.sync.dma_start(out=dw_t[:, :], in_=dif_w[:, :])
            kb_t = cpool.tile([128, 1], F32)
            nc.sync.dma_start(out=kb_t[:, :], in_=kbias[:, :])
            sb_t = cpool.tile([128, 2 * D], F16)
            nc.sync.dma_start(out=sb_t[:, :], in_=sbt[:, :])
            # Two persistent 128-row bucket tiles: rows 3-127 zeroed once;
            # rows 0-2 re-filled by each chunk DMA (keeps the diff matmul
            # K=128 for HAM full-row activity).
            bts = []
            for i in range(2):
                btp = cpool.tile([128, BCHUNK], F16, name=f"btp{i}")
                nc.vector.memset(btp[:, :], 0.0)
                bts.append(btp)

            # HAM warm-up primer: ~16 gapless dummy matmuls release the PE
            # clock throttle (4096-cycle fully-busy window required); the
            # main loop's sub-us gaps then never re-throttle it.
            prime_w = cpool.tile([128, 128], F16)
            nc.vector.memset(prime_w[:, :], 0.0)
            prime_x = cpool.tile([128, 512], F16)
            nc.vector.memset(prime_x[:, :], 0.0)
            for i in range(16):
                prime_ps = psgpool.tile(
                    [128, 2 * GROUP], F32, tag="ps", name=f"prime{i}"
                )
                nc.tensor.matmul(
                    prime_ps[:, 0:512], prime_w[:, :], prime_x[:, :],
                    start=True, stop=True,
                )

            # Software pipeline: one-hot production runs 2 groups ahead of
            # the gather matmuls so the PE never waits on the ACT Relu.
            bt = None
            psds = {}
            ohs = {}
            ress = {}
            ftiles = {}   # region idx -> (tile, region meta)

            def flt_view(res_tile, lo_f, n_f):
                return res_tile[:, lo_f : lo_f + n_f]

            for step in range(len(sched) + 2):
                if step < len(sched):
                    ri, gl = sched[step]
                    base, n, stripe, ngr, parts = REGIONS[ri]
                    slot0 = 512 * step
                    if slot0 % BCHUNK == 0:
                        bt = bts[(slot0 // BCHUNK) % 2]
                        csz = min(BCHUNK, NSLOT - slot0)
                        nc.sync.dma_start(
                            out=bt[0:BROWS, 0:csz],
                            in_=brow[:, slot0 : slot0 + csz],
                        )
                    if gl == 0:
                        # region feature load: partition p holds `stripe`
                        # consecutive samples (contiguous DRAM run).
                        ft = fpool.tile([128, SUPER], F32, tag="ft")
                        nfl = n * D // 128  # floats per partition if full
                        if parts == 128:
                            nc.sync.dma_start(
                                out=ft[:, 0:nfl],
                                in_=feat[base : base + n, :].rearrange(
                                    "(p j) d -> p (j d)", j=stripe
                                ),
                            )
                        else:
                            nc.sync.dma_start(
                                out=ft[0:parts, 0 : stripe * D],
                                in_=feat[base : base + n, :].rearrange(
                                    "(p j) d -> p (j d)", j=stripe
                                ),
                            )
                        ftiles[ri] = ft
                    boff = slot0 % BCHUNK
                    psd = psdpool.tile([128, GROUP], F32, tag="psd")
                    nc.tensor.matmul(
                        psd[:, :],
                        dw_t[:, :],
                        bt[:, boff : boff + GROUP],
                        start=True,
                        stop=True,
                    )
                    psds[step] = psd
                if 1 <= step <= len(sched):
                    g = step - 1
                    oh = opool.tile([128, GROUP], F16, tag="oh")
                    nc.scalar.activation(
                        oh[:, :],
                        psds.pop(g)[:, :],
                        mybir.ActivationFunctionType.Relu,
                        bias=kb_t[:, :],
                        scale=-1.0,
                    )
                    ohs[g] = oh
                if step >= 2:
                    g = step - 2
                    ri, gl = sched[g]
                    base, n, stripe, ngr, parts = REGIONS[ri]
                    oh = ohs.pop(g)
                    ft = ftiles[ri]
                    nt = GROUP // 128
                    # psum tile t: cols [t*256,+128) = Sg, [+128,+128) = Bg
                    ps = psgpool.tile([128, 2 * GROUP], F32, tag="ps")
                    for t in range(nt):
                        nc.tensor.matmul(
                            ps[:, t * 256 : (t + 1) * 256],
                            oh[:, t * 128 : (t + 1) * 128],
                            sb_t[:, :],
                            start=True,
                            stop=True,
                        )
                    ps3 = ps[:, :].rearrange("p (t c) -> p t c", c=256)
                    f3 = ft[:, gl * 512 : (gl + 1) * 512].rearrange(
                        "p (t d) -> p t d", d=128
                    )
                    tmp = mpool.tile([128, GROUP], F32, tag="tmp")
                    t3 = tmp[:, :].rearrange("p (t d) -> p t d", d=128)
                    nc.vector.tensor_tensor(
                        t3, f3, ps3[:, :, 0:128], mybir.AluOpType.mult
                    )
                    # Pair two groups per store DMA (4KB descriptors) to
                    # halve the scalar-sequencer trigger load.
                    pg = gl % 2
                    if pg == 0:
                        res2 = rpool.tile([128, 2 * GROUP], F32, tag="res")
                        ress[g] = res2
                        if gl + 1 < ngr:
                            ress[g + 1] = res2
                    res2 = ress.pop(g)
                    r3 = res2[:, pg * GROUP : (pg + 1) * GROUP].rearrange(
                        "p (t d) -> p t d", d=128
                    )
                    nc.vector.tensor_tensor(
                        r3, t3, ps3[:, :, 128:256], mybir.AluOpType.add
                    )
                    if parts != 128:
                        # 36-sample tail: store only the 9 real partitions.
                        nc.scalar.dma_start(
                            out=outp[base : base + n, :].rearrange(
                                "(p j) d -> p (j d)", j=stripe
                            ),
                            in_=res2[0:parts, 0 : stripe * D],
                        )
                    elif pg == 1:
                        # pair store: rows base + stripe*p + (4*(gl-1) + jj),
                        # jj in [0,8) -> one contiguous 4KB run per partition
                        j0 = 4 * (gl - 1)
                        dst = outp[base : base + n, :].rearrange(
                            "(p j) d -> p j d", j=stripe
                        )[:, j0 : j0 + 8, :]
                        nc.scalar.dma_start(
                            out=dst,
                            in_=res2[:, :].rearrange("p (j d) -> p j d", d=128),
                        )
    return nc


_CACHED_NC = None


def _get_program():
    global _CACHED_NC
    if _CACHED_NC is None:
        _CACHED_NC = build_program()
        split_waits(_CACHED_NC)
    return _CACHED_NC


def _host_tables(m1, v1, m2, v2):
    pos = v1 > 0
    v1_safe = np.where(pos, v1, np.float32(1.0)).astype(np.float32)
    factor = np.clip(v2 / v1_safe, np.float32(CLIP_MIN), np.float32(CLIP_MAX))
    s = np.sqrt(factor.astype(np.float32)).astype(np.float32)
    s = np.where(pos, s, np.float32(1.0)).astype(np.float32)
    b = np.where(pos, m2 - m1 * s, np.float32(0.0)).astype(np.float32)
    return s, b


def _slot_perm():
    """Map slot index -> per-core sample index (or -1 for pad).

    Slot (group q, t, p) = q*512 + t*128 + p holds sample
    region.base + stripe*p + 4*gl + t  so that one-hot column t*128+m
    lands on psum partition m = the sample striped into ft partition m.
    """
    idx = np.full(NSLOT, -1, dtype=np.int64)
    q = 0
    for base, n, stripe, ngr, parts in REGIONS:
        for gl in range(ngr):
            for t in range(4):
                p = np.arange(parts)
                s = stripe * p + 4 * gl + t
                ok = s < n
                idx[q * 512 + t * 128 + p[ok]] = base + s[ok]
            q += 1
    return idx


_SLOT_IDX = _slot_perm()


def make_inputs(features, bucketsf, sbt):
    """Build per-core input maps (host-side shard + slot permute)."""
    k = np.arange(128, dtype=np.float64)
    dif_w = np.zeros((128, 128), dtype=np.float16)
    dif_w[0] = -2.0 * k
    dif_w[1] = 1.0
    dif_w[2] = 1.0
    # oh_k = Relu(-psd + (1 - k^2)) = Relu(1 - (b-k)^2): exact one-hot
    kbias = (1.0 - k * k).astype(np.float32).reshape(128, 1)

    b = bucketsf.astype(np.float64)
    b2 = b * b
    b2hi = b2.astype(np.float16)
    b2lo = (b2 - b2hi.astype(np.float64)).astype(np.float16)

    valid = _SLOT_IDX >= 0
    safe_idx = np.where(valid, _SLOT_IDX, 0)
    in_maps = []
    for c in range(NCORES):
        lo = c * PER
        b_c = np.empty((BROWS, NSLOT), dtype=np.float16)
        # pad slots: b=-1 -> (b-k)^2 = (k+1)^2 >= 1 -> one-hot zero
        b_c[0] = np.where(valid, b[lo + safe_idx], -1.0)
        b_c[1] = np.where(valid, b2hi[lo + safe_idx], 1.0)
        b_c[2] = np.where(valid, b2lo[lo + safe_idx], 0.0)
        in_maps.append(
            {
                "feat": features[lo : lo + PER],
                "brow": b_c,
                "dif_w": dif_w,
                "kbias": kbias,
                "sbt": sbt,
            }
        )
    return in_maps


def kernel(
    features,
    buckets,
    running_mean_last_epoch,
    running_var_last_epoch,
    smoothed_mean_last_epoch,
    smoothed_var_last_epoch,
    epoch,
):
    global LAST_RESULTS
    features = np.ascontiguousarray(np.asarray(features, dtype=np.float32))
    buckets = np.asarray(buckets)
    m1 = np.asarray(running_mean_last_epoch, dtype=np.float32)
    v1 = np.asarray(running_var_last_epoch, dtype=np.float32)
    m2 = np.asarray(smoothed_mean_last_epoch, dtype=np.float32)
    v2 = np.asarray(smoothed_var_last_epoch, dtype=np.float32)
    epoch = int(np.asarray(epoch))

    if epoch < 1:  # START_SMOOTH
        return features.copy()

    s, b = _host_tables(m1, v1, m2, v2)
    sb = np.concatenate([s, b], axis=1)  # [NB, 256] f32
    sbt = np.zeros((128, 2 * D), dtype=np.float16)
    sbt[:NB] = sb.astype(np.float16)
    in_maps = make_inputs(features, buckets.astype(np.float64), sbt)

    nc = _get_program()
    LAST_RESULTS = run_bass_kernel_spmd(nc, in_maps, list(range(NCORES)))
    out = np.empty((N, D), dtype=np.float32)
    for c in range(NCORES):
        out[c * PER : (c + 1) * PER] = LAST_RESULTS.results[c]["outp"]
    return out


# revision 11
# speedup vs baseline: 1.0959x; 1.0959x over previous
"""FDS smooth kernel for Trainium2 (8 NeuronCores, data-parallel).

Math: out[i,:] = features[i,:] * S[b_i,:] + B[b_i,:]
  S = sqrt(clip(v2/v1, 0.1, 10))  (1.0 where v1 <= 0)
  B = m2 - m1*S                   (0.0 where v1 <= 0)
S/B are tiny [100,128] per-bucket tables precomputed on host (f16) and
replicated to every core.  Per 512-sample compute group on device:
  PE:   diff2[k,i] = (b_i - k)^2 via K=128 f16 matmul (exact hi/lo b^2)
  ACT:  oh = Relu(1 - diff2)  -> exact one-hot, f16
  PE:   per 128-sample tile: psum = oh_tile.T @ [S||B]  (f16 matmul)
  DVE:  out = f * Sg + Bg   (two tensor_tensor ops on strided PSUM views)
  DMA:  loads on sync (SP HWDGE ring), stores on scalar (ACT ring)

HBM layout per core (62500 samples, zero padding traffic):
  30 supers x 2048 samples, stripe 16/partition -> 8KB descriptors
  1 mini of 1024, stripe 8 -> 4KB descriptors
  1 tail of 36, stripe 4 on partitions 0..8
Stores are issued per 1024-sample pair (4KB descriptors).
"""

import os
import sys
import types

import bass_rust
import ml_dtypes
import numpy as np

import concourse.bass as bass
import concourse.mybir as mybir
from concourse.bass_types import AP
from concourse.bass_utils import run_bass_kernel_spmd
from concourse.tile import TileContext

# This walrus build accepts at most one semaphore wait per instruction.
WAIT_LIMIT = 1


def split_waits(nc, maxw=WAIT_LIMIT):
    """Move excess sem waits onto standalone same-engine carriers
    inserted immediately before the over-limit instruction."""
    n = 0
    for fn in nc.m.functions:
        for blk in fn.blocks:
            insts = blk.instructions
            if not any(
                i.sync_info is not None and len(i.sync_info.on_wait) > maxw
                for i in insts
            ):
                continue
            newl = []
            for ins in insts:
                si = ins.sync_info
                if si is not None and len(si.on_wait) > maxw:
                    waits = list(si.on_wait)
                    extra, keep = waits[:-maxw], waits[-maxw:]
                    while extra:
                        chunk, extra = extra[:maxw], extra[maxw:]
                        # EventSemaphore = sequencer-level wait carrier that
                        # does NOT flush the engine pipeline (a Drain would).
                        d = bass_rust.InstEventSemaphore(
                            name=f"WSPL-{nc.next_id()}", ins=[], outs=[]
                        )
                        d.engine = ins.engine
                        d.sync_info = mybir.SyncInfo(on_wait=chunk, on_update=[])
                        newl.append(d)
                        n += 1
                    ins.sync_info = mybir.SyncInfo(
                        on_wait=keep, on_update=list(si.on_update)
                    )
                newl.append(ins)
            blk.instructions = newl
    return n


N = 500_000
D = 128
NB = 100          # buckets
NCORES = 8
CLIP_MIN = 0.1
CLIP_MAX = 10.0

PER = N // NCORES             # 62500 samples per core
GROUP = 512                   # samples per compute group
SUPER = 2048                  # samples per feature-load DMA (1MB f32)
NSUPER = PER // SUPER         # 30 full supers
MINI = 1024                   # one mid chunk
TAILN = PER - NSUPER * SUPER - MINI   # 36
TAILP = TAILN // 4            # 9 partitions used by the tail group
BROWS = 3                     # b, hi(b^2), lo(b^2)
BCHUNK = 4096                 # one-hot slots per bucket-row DMA chunk

F32 = mybir.dt.float32
F16 = mybir.dt.float16

LAST_RESULTS = None           # test harness reads exec_time_ns off this


def _ensure_ntff_shim():
    """If BASS_TRACE is set but the image's antenv lacks axon_hooks,
    run_bass_kernel_spmd(trace=True) would die on import.  Provide the
    hook (via trn_agent_boot's ctypes path) or a None stub."""
    try:
        import antenv.axon_hooks  # noqa: F401
        return
    except ImportError:
        pass
    hook = None
    try:
        from trn_agent_boot.trn_boot import _ntff_profile_via_ctypes

        hook = _ntff_profile_via_ctypes("/opt/axon/libaxon_pjrt.so")
    except Exception:
        hook = None
    mod = types.ModuleType("antenv.axon_hooks")
    mod.get_axon_ntff_profile_hook = lambda: hook
    mod.set_axon_ntff_profile_hook = lambda h: None
    sys.modules["antenv.axon_hooks"] = mod
    try:
        import concourse.bass_utils as _bu

        _bu.upload_artifacts = lambda tmpdir: f"local://{tmpdir}"
    except Exception:
        pass


_ensure_ntff_shim()


def _regions():
    """Per-core sample regions: (base, nsamples, stripe, ngroups, parts)."""
    regs = []
    for s in range(NSUPER):
        regs.append((s * SUPER, SUPER, 16, SUPER // GROUP, 128))
    regs.append((NSUPER * SUPER, MINI, 8, MINI // GROUP, 128))
    regs.append((NSUPER * SUPER + MINI, TAILN, 4, 1, TAILP))
    return regs


REGIONS = _regions()
NGROUPS = sum(r[3] for r in REGIONS)          # 123
NSLOT = NGROUPS * GROUP                       # 62976 one-hot slots


def build_program():
    nc = bass.Bass("TRN2", debug=False)

    feat = nc.dram_tensor("feat", [PER, D], F32, kind="ExternalInput")
    # rows: b, hi(b^2), lo(b^2)  (f16), in permuted slot order
    brow = nc.dram_tensor("brow", [BROWS, NSLOT], F16, kind="ExternalInput")
    # cols k: psd[k,i] = -2k*b + b2hi + b2lo ; the k^2 term rides in the
    # per-partition activation bias (1 - k^2), keeping bt rows 3-127 zero.
    dif_w = nc.dram_tensor("dif_w", [128, 128], F16, kind="ExternalInput")
    kbias = nc.dram_tensor("kbias", [128, 1], F32, kind="ExternalInput")
    # [S||B] f16; rows 100-127 zero.
    sbt = nc.dram_tensor("sbt", [128, 2 * D], F16, kind="ExternalInput")
    outp = nc.dram_tensor("outp", [PER, D], F32, kind="ExternalOutput")

    # flat group schedule: (region_idx, local_group)
    sched = []
    for ri, (base, n, stripe, ngr, parts) in enumerate(REGIONS):
        for gl in range(ngr):
            sched.append((ri, gl))

    with TileContext(nc) as tc:
        with (
            tc.tile_pool(name="const", bufs=1) as cpool,
            tc.tile_pool(name="fin", bufs=3) as fpool,
            tc.tile_pool(name="bin", bufs=2) as bpool,
            tc.tile_pool(name="onehot", bufs=3) as opool,
            tc.tile_pool(name="mid", bufs=2) as mpool,
            tc.tile_pool(name="res", bufs=4) as rpool,
            tc.tile_pool(name="psd", bufs=3, space="PSUM") as psdpool,
            tc.tile_pool(name="psg", bufs=2, space="PSUM") as psgpool,
        ):
            dw_t = cpool.tile([128, 128], F16)
            nc.sync.dma_start(out=dw_t[:, :], in_=dif_w[:, :])
            kb_t = cpool.tile([128, 1], F32)
            nc.sync.dma_start(out=kb_t[:, :], in_=kbias[:, :])
            sb_t = cpool.tile([128, 2 * D], F16)
            nc.sync.dma_start(out=sb_t[:, :], in_=sbt[:, :])
            # HAM warm-up primer: ~16 gapless dummy matmuls release the PE
            # clock throttle (4096-cycle fully-busy window required); the
            # main loop's sub-us gaps then never re-throttle it.
            prime_w = cpool.tile([128, 128], F16)
            nc.vector.memset(prime_w[:, :], 0.0)
            prime_x = cpool.tile([128, 512], F16)
            nc.vector.memset(prime_x[:, :], 0.0)
            # Two persistent 128-row bucket tiles: rows 3-127 zeroed once
            # (split across DVE and GpSimd so neither blocks startup);
            # rows 0-2 re-filled by each chunk DMA (keeps the diff matmul
            # K=128 for HAM full-row activity).
            bts = []
            for i in range(2):
                btp = cpool.tile([128, BCHUNK], F16, name=f"btp{i}")
                (nc.vector if i == 0 else nc.gpsimd).memset(btp[:, :], 0.0)
                bts.append(btp)
            for i in range(16):
                prime_ps = psgpool.tile(
                    [128, 2 * GROUP], F32, tag="ps", name=f"prime{i}"
                )
                nc.tensor.matmul(
                    prime_ps[:, 0:512], prime_w[:, :], prime_x[:, :],
                    start=True, stop=True,
                )

            # Software pipeline: one-hot production runs 2 groups ahead of
            # the gather matmuls so the PE never waits on the ACT Relu.
            psds = {}
            ohs = {}
            ress = {}
            ftiles = {}   # region idx -> feature tile

            def issue_bchunk(ci):
                bt = bts[ci % 2]
                s0 = ci * BCHUNK
                csz = min(BCHUNK, NSLOT - s0)
                nc.sync.dma_start(
                    out=bt[0:BROWS, 0:csz], in_=brow[:, s0 : s0 + csz]
                )

            def issue_ft(ri):
                # region feature load: partition p holds `stripe`
                # consecutive samples (contiguous DRAM run).
                base, n, stripe, ngr, parts = REGIONS[ri]
                ft = fpool.tile([128, SUPER], F32, tag="ft")
                nfl = stripe * D  # floats per partition
                nc.sync.dma_start(
                    out=ft[0:parts, 0:nfl],
                    in_=feat[base : base + n, :].rearrange(
                        "(p j) d -> p (j d)", j=stripe
                    ),
                )
                ftiles[ri] = ft

            # Startup order on the SP ring: ft0 first so the DMA engines
            # stream from t~0; b0 after ft1 so its wait on the btp0
            # memset hides under the feature transfers.
            issue_ft(0)
            issue_ft(1)
            issue_bchunk(0)
            issue_ft(2)

            for step in range(len(sched) + 2):
                if step < len(sched):
                    ri, gl = sched[step]
                    base, n, stripe, ngr, parts = REGIONS[ri]
                    slot0 = 512 * step
                    if slot0 % BCHUNK == 0 and slot0 > 0:
                        issue_bchunk(slot0 // BCHUNK)
                    if gl == 0 and ri >= 3:
                        issue_ft(ri)
                    bt = bts[(slot0 // BCHUNK) % 2]
                    boff = slot0 % BCHUNK
                    psd = psdpool.tile([128, GROUP], F32, tag="psd")
                    nc.tensor.matmul(
                        psd[:, :],
                        dw_t[:, :],
                        bt[:, boff : boff + GROUP],
                        start=True,
                        stop=True,
                    )
                    psds[step] = psd
                if 1 <= step <= len(sched):
                    g = step - 1
                    oh = opool.tile([128, GROUP], F16, tag="oh")
                    nc.scalar.activation(
                        oh[:, :],
                        psds.pop(g)[:, :],
                        mybir.ActivationFunctionType.Relu,
                        bias=kb_t[:, :],
                        scale=-1.0,
                    )
                    ohs[g] = oh
                if step >= 2:
                    g = step - 2
                    ri, gl = sched[g]
                    base, n, stripe, ngr, parts = REGIONS[ri]
                    oh = ohs.pop(g)
                    ft = ftiles[ri]
                    nt = GROUP // 128
                    # psum tile t: cols [t*256,+128) = Sg, [+128,+128) = Bg
                    ps = psgpool.tile([128, 2 * GROUP], F32, tag="ps")
                    for t in range(nt):
                        nc.tensor.matmul(
                            ps[:, t * 256 : (t + 1) * 256],
                            oh[:, t * 128 : (t + 1) * 128],
                            sb_t[:, :],
                            start=True,
                            stop=True,
                        )
                    ps3 = ps[:, :].rearrange("p (t c) -> p t c", c=256)
                    f3 = ft[:, gl * 512 : (gl + 1) * 512].rearrange(
                        "p (t d) -> p t d", d=128
                    )
                    tmp = mpool.tile([128, GROUP], F32, tag="tmp")
                    t3 = tmp[:, :].rearrange("p (t d) -> p t d", d=128)
                    nc.vector.tensor_tensor(
                        t3, f3, ps3[:, :, 0:128], mybir.AluOpType.mult
                    )
                    # Pair two groups per store DMA (4KB descriptors) to
                    # halve the scalar-sequencer trigger load.
                    pg = gl % 2
                    if pg == 0:
                        res2 = rpool.tile([128, 2 * GROUP], F32, tag="res")
                        ress[g] = res2
                        if gl + 1 < ngr:
                            ress[g + 1] = res2
                    res2 = ress.pop(g)
                    r3 = res2[:, pg * GROUP : (pg + 1) * GROUP].rearrange(
                        "p (t d) -> p t d", d=128
                    )
                    nc.vector.tensor_tensor(
                        r3, t3, ps3[:, :, 128:256], mybir.AluOpType.add
                    )
                    if parts != 128:
                        # 36-sample tail: store only the 9 real partitions.
                        nc.scalar.dma_start(
                            out=outp[base : base + n, :].rearrange(
                                "(p j) d -> p (j d)", j=stripe
                            ),
                            in_=res2[0:parts, 0 : stripe * D],
                        )
                    elif pg == 1:
                        # pair store: rows base + stripe*p + (4*(gl-1) + jj),
                        # jj in [0,8) -> one contiguous 4KB run per partition
                        j0 = 4 * (gl - 1)
                        dst = outp[base : base + n, :].rearrange(
                            "(p j) d -> p j d", j=stripe
                        )[:, j0 : j0 + 8, :]
                        nc.scalar.dma_start(
                            out=dst,
                            in_=res2[:, :].rearrange("p (j d) -> p j d", d=128),
                        )
    return nc


_CACHED_NC = None


def _get_program():
    global _CACHED_NC
    if _CACHED_NC is None:
        _CACHED_NC = build_program()
        split_waits(_CACHED_NC)
    return _CACHED_NC


def _host_tables(m1, v1, m2, v2):
    pos = v1 > 0
    v1_safe = np.where(pos, v1, np.float32(1.0)).astype(np.float32)
    factor = np.clip(v2 / v1_safe, np.float32(CLIP_MIN), np.float32(CLIP_MAX))
    s = np.sqrt(factor.astype(np.float32)).astype(np.float32)
    s = np.where(pos, s, np.float32(1.0)).astype(np.float32)
    b = np.where(pos, m2 - m1 * s, np.float32(0.0)).astype(np.float32)
    return s, b


def _slot_perm():
    """Map slot index -> per-core sample index (or -1 for pad).

    Slot (group q, t, p) = q*512 + t*128 + p holds sample
    region.base + stripe*p + 4*gl + t  so that one-hot column t*128+m
    lands on psum partition m = the sample striped into ft partition m.
    """
    idx = np.full(NSLOT, -1, dtype=np.int64)
    q = 0
    for base, n, stripe, ngr, parts in REGIONS:
        for gl in range(ngr):
            for t in range(4):
                p = np.arange(parts)
                s = stripe * p + 4 * gl + t
                ok = s < n
                idx[q * 512 + t * 128 + p[ok]] = base + s[ok]
            q += 1
    return idx


_SLOT_IDX = _slot_perm()


def make_inputs(features, bucketsf, sbt):
    """Build per-core input maps (host-side shard + slot permute)."""
    k = np.arange(128, dtype=np.float64)
    dif_w = np.zeros((128, 128), dtype=np.float16)
    dif_w[0] = -2.0 * k
    dif_w[1] = 1.0
    dif_w[2] = 1.0
    # oh_k = Relu(-psd + (1 - k^2)) = Relu(1 - (b-k)^2): exact one-hot
    kbias = (1.0 - k * k).astype(np.float32).reshape(128, 1)

    b = bucketsf.astype(np.float64)
    b2 = b * b
    b2hi = b2.astype(np.float16)
    b2lo = (b2 - b2hi.astype(np.float64)).astype(np.float16)

    valid = _SLOT_IDX >= 0
    safe_idx = np.where(valid, _SLOT_IDX, 0)
    in_maps = []
    for c in range(NCORES):
        lo = c * PER
        b_c = np.empty((BROWS, NSLOT), dtype=np.float16)
        # pad slots: b=-1 -> (b-k)^2 = (k+1)^2 >= 1 -> one-hot zero
        b_c[0] = np.where(valid, b[lo + safe_idx], -1.0)
        b_c[1] = np.where(valid, b2hi[lo + safe_idx], 1.0)
        b_c[2] = np.where(valid, b2lo[lo + safe_idx], 0.0)
        in_maps.append(
            {
                "feat": features[lo : lo + PER],
                "brow": b_c,
                "dif_w": dif_w,
                "kbias": kbias,
                "sbt": sbt,
            }
        )
    return in_maps


def kernel(
    features,
    buckets,
    running_mean_last_epoch,
    running_var_last_epoch,
    smoothed_mean_last_epoch,
    smoothed_var_last_epoch,
    epoch,
):
    global LAST_RESULTS
    features = np.ascontiguousarray(np.asarray(features, dtype=np.float32))
    buckets = np.asarray(buckets)
    m1 = np.asarray(running_mean_last_epoch, dtype=np.float32)
    v1 = np.asarray(running_var_last_epoch, dtype=np.float32)
    m2 = np.asarray(smoothed_mean_last_epoch, dtype=np.float32)
    v2 = np.asarray(smoothed_var_last_epoch, dtype=np.float32)
    epoch = int(np.asarray(epoch))

    if epoch < 1:  # START_SMOOTH
        return features.copy()

    s, b = _host_tables(m1, v1, m2, v2)
    sb = np.concatenate([s, b], axis=1)  # [NB, 256] f32
    sbt = np.zeros((128, 2 * D), dtype=np.float16)
    sbt[:NB] = sb.astype(np.float16)
    in_maps = make_inputs(features, buckets.astype(np.float64), sbt)

    nc = _get_program()
    LAST_RESULTS = run_bass_kernel_spmd(nc, in_maps, list(range(NCORES)))
    out = np.empty((N, D), dtype=np.float32)
    for c in range(NCORES):
        out[c * PER : (c + 1) * PER] = LAST_RESULTS.results[c]["outp"]
    return out
